# revision 47
# baseline (speedup 1.0000x reference)
"""Trainium2 Bass kernel for nn_DisLoss (prototype EMA + masked pairwise exp-sim loss).

Strategy (8 NeuronCores, SPMD, one compiled program + per-core data rotation):
  - The loss is invariant under any permutation of the class axis, and rows of
    the [C,C] logits matrix are independent.  The host therefore RE-LABELS the
    class space per core: the 1024 "chain lane" classes (the distinct labels
    plus untouched filler classes, whose zero-feature chains are identity maps)
    are ordered so that core r's tile 0 holds exactly its own 128 row-classes,
    and the 7168 untouched "K classes" are rotated per core so that its own 896
    K rows are columns 0..895 of its ptT copy (lhsT == ptT slice, no extra
    input).
  - The host pre-transposes the K-class prototype columns (ptTK, fp16), so the
    [rows x cols] matmul + exp stream starts as soon as the first column chunk
    lands (~12us), fully overlapping the EMA chain.  Chunk DMAs are serialized
    per HWDGE ring (the DGE round-robins packets of all outstanding descriptors,
    so without explicit ordering chunk 0 would land LAST).
  - The EMA chain (deferred-normalization form: v <- v + ||v||*f per round,
    normalize once at the end) runs replicated on the DVE (norms via mul +
    reduce) so the ACT engine stays dedicated to the exp stream.  Chain output
    tiles are xbar-DMA-transposed into the LAST 1024 columns of ptT and into
    the block-7 lhsT, which the matmul loop consumes last.
  - exp writes fp16 (max value exp(10) < fp16 max) and the per-row sums are
    accumulated by the DVE (tensor_scalar at 4x rate with accum_out), keeping
    the ~0.3us/group ACT accumulator-read off the critical engine.
  - Diagonal masking: row i's self-logit is ||row_i||^2/T; the host sends the
    exact fp16 squared norms and the device subtracts exp(10*n) per row before
    the final Ln.  Each core reduces its 1024 rows to one scalar; the host
    sums 8 scalars.
"""

import math
from contextlib import ExitStack

import numpy as np

import types as _pytypes

import bass_rust as _bass_rust
import concourse.bass as bass
import concourse.mybir as mybir
import concourse.tile as tile
from concourse import bacc
from concourse.bass_utils import run_bass_kernel_spmd
from concourse.hw_specs import get_activation_tables
from concourse.tile_rust import add_dep_helper

ACT_SET = "natural_log_exp_and_others"  # contains Exp + Ln


def _pin_act_tables(nc):
    """Force all activations onto one table set (avoids ~2.7us set reloads)."""

    def patched(self):
        has_act = any(
            isinstance(i, mybir.InstActivation)
            for b in self.main_func.blocks
            for i in b.instructions
        )
        if not has_act:
            return
        tables = [
            (name, fns if name == ACT_SET else type(fns)())
            for name, fns in get_activation_tables(self.m.arch).items()
        ]
        _bass_rust.insert_act_table_loads(self, tables)

    nc.insert_act_table_loads = _pytypes.MethodType(patched, nc)


P = 128
C = 8192
D = 256
B = 1024
NCORES = 8
CPC = C // NCORES          # row classes per core (1024)
NT = CPC // P              # chain lane tiles (8); chain lanes == CPC
KC = CPC - P               # own K rows per core (896) == ptT cols 0..895
KH = C - CPC               # K-class columns (7168)
NB = CPC // P              # own row blocks (8)
GW = 2048                  # psum group width (4 banks, double-buffered)
NG = C // GW               # col groups (4)
TEMP = 0.1
BASE_TEMP = 0.1
EXP10 = math.exp(1.0 / TEMP)

F32 = mybir.dt.float32
F16 = mybir.dt.float16
I32 = mybir.dt.int32
AX = mybir.AxisListType.X
ADD = mybir.AluOpType.add
SUB = mybir.AluOpType.subtract
MULT = mybir.AluOpType.mult
EXPF = mybir.ActivationFunctionType.Exp
LNF = mybir.ActivationFunctionType.Ln


def _ins(x):
    return getattr(x, "ins", x)


def _chain_dma(prev, cur):
    if prev is not None:
        add_dep_helper(_ins(cur), _ins(prev), sync=True, reason="serialize ring")
    return cur


def build_program(NFT, R, RT, fo_list):
    """One SPMD Bass program; all shape-relevant values are core-invariant."""
    nc = bacc.Bacc("TRN2", target_bir_lowering=False, debug=False, num_devices=NCORES)
    _pin_act_tables(nc)

    # ufg: [chain-lane proto rows (NT tiles); per-round feature rows (NFT tiles)]
    ufg_d = nc.declare_dram_parameter("ufg", [P, NT + NFT, D], F16, isOutput=False)
    # per-core-rotated K columns, pre-transposed: ptTK[h][d, j] = proto16[krot[j], h*128+d]
    ptTK_d = nc.declare_dram_parameter("ptTK", [2, P, KH], F16, isOutput=False)
    # exact fp16 squared norms of own rows (block 7 slot = 1.0)
    n16_d = nc.declare_dram_parameter("n16", [P, NB], F32, isOutput=False)
    # per-row ln((rowsum - exp(10*n))/(C-1)); host sums across rows and cores
    out_d = nc.declare_dram_parameter("partial", [P, NB], F32, isOutput=True)

    with tile.TileContext(nc) as tc:
        with ExitStack() as ctx:
            aux = ctx.enter_context(tc.tile_pool(name="aux", bufs=1))
            chainp = ctx.enter_context(tc.tile_pool(name="chain", bufs=1))
            psp = ctx.enter_context(tc.tile_pool(name="ps", bufs=2, space="PSUM"))
            bigp = ctx.enter_context(tc.tile_pool(name="big", bufs=1))
            scrp = ctx.enter_context(tc.tile_pool(name="scr", bufs=2))

            ones_sb = aux.tile([P, 1], F32)
            nc.vector.memset(ones_sb[:], 1.0)

            # ---- input DMAs ----
            # Ring bandwidths (measured): gpsimd ~200 GB/s, scalar ~95, sync
            # ~48.  The DGE round-robins outstanding descriptors, so the
            # gpsimd ring is explicitly chained in consumption order.  The
            # scalar ring only gets UNCHAINED kicks (a chained kick's wait
            # would block the ACT engine mid-stream).
            ptT = [bigp.tile([P, C], F16, name=f"ptT{h}") for h in range(2)]
            n16 = aux.tile([P, NB], F32)
            nc.sync.dma_start(n16[:], n16_d[:])
            ufg = chainp.tile([P, NT + NFT, D], F16)
            HG = GW // 2
            # scalar ring kicks earliest (short ACT preamble), fire-and-forget:
            # the first 1024-col half plus col group 2
            nc.scalar.dma_start(ptT[0][:, 0:HG], ptTK_d[0, :, 0:HG])
            nc.scalar.dma_start(ptT[1][:, 0:HG], ptTK_d[1, :, 0:HG])
            nc.scalar.dma_start(ptT[0][:, 2 * GW : 3 * GW], ptTK_d[0, :, 2 * GW : 3 * GW])
            nc.scalar.dma_start(ptT[1][:, 2 * GW : 3 * GW], ptTK_d[1, :, 2 * GW : 3 * GW])
            # force the (single) activation table set to load while DMAs run
            # (after the scalar-ring kicks -- the ACT queue is strict FIFO)
            dummy = aux.tile([1, 1], F32)
            nc.scalar.activation(out=dummy[:], in_=ones_sb[0:1, 0:1], func=LNF)
            # gpsimd ring (fast), FIFO order = consumption order: rest of group
            # 0, chain inputs (longest downstream path: chain -> 18 slow
            # sync-ring transposes), group 1, group 3's K part
            nc.gpsimd.dma_start(ptT[0][:, HG:GW], ptTK_d[0, :, HG:GW])
            nc.gpsimd.dma_start(ptT[1][:, HG:GW], ptTK_d[1, :, HG:GW])
            nc.gpsimd.dma_start(ufg[:, :, :], ufg_d[:, :, :])
            nc.gpsimd.dma_start(ptT[0][:, GW : 2 * GW], ptTK_d[0, :, GW : 2 * GW])
            nc.gpsimd.dma_start(ptT[1][:, GW : 2 * GW], ptTK_d[1, :, GW : 2 * GW])
            nc.gpsimd.dma_start(ptT[0][:, 3 * GW : KH], ptTK_d[0, :, 3 * GW : KH])
            nc.gpsimd.dma_start(ptT[1][:, 3 * GW : KH], ptTK_d[1, :, 3 * GW : KH])

            # ---- EMA chain, DVE-resident (ACT only for tiny Ln/Exp on norms) ----
            uf = ufg[:, 0:NT, :]
            fg = ufg[:, NT:, :]
            u = chainp.tile([P, NT, D], F32)
            sq = chainp.tile([P, D], F32)
            n2 = chainp.tile([P, NT], F32)
            rinv = chainp.tile([P, NT], F32)
            tmp = chainp.tile([P, NT], F32)
            nrm = chainp.tile([P, NT], F32)
            magic = chainp.tile([P, NT], I32)
            nc.vector.memset(magic[:], 0x5F3759DF)
            fscl = chainp.tile([P, D], F32)
            u16 = chainp.tile([P, NT, D], F16)
            lhsT7 = [aux.tile([P, P], F16, name=f"lhsT7{h}") for h in range(2)]

            def rsqrt_dve(lo, hi):
                """rinv[:, lo:hi] = 1/sqrt(n2[:, lo:hi]) entirely on the DVE
                (fast-inverse-sqrt bit trick + 2 Newton iterations, rel err
                ~4e-6) so the chain never touches the ACT engine."""
                a = lambda t: t[:, lo:hi]
                nc.vector.tensor_scalar(
                    out=a(tmp).bitcast(I32), in0=a(n2).bitcast(I32),
                    scalar1=1, scalar2=None, op0=mybir.AluOpType.logical_shift_right,
                )
                nc.vector.tensor_sub(a(rinv).bitcast(I32), a(magic), a(tmp).bitcast(I32))
                for _ in range(2):
                    nc.vector.tensor_mul(a(tmp), a(rinv), a(rinv))
                    nc.vector.tensor_mul(a(tmp), a(tmp), a(n2))
                    nc.vector.tensor_scalar(
                        out=a(tmp), in0=a(tmp),
                        scalar1=-0.5, scalar2=1.5, op0=MULT, op1=ADD,
                    )
                    nc.vector.tensor_mul(a(rinv), a(rinv), a(tmp))

            # warm the PE's HAM clock gate while the first column chunks land:
            # must be FAT matmuls (N=512) -- tiny ones don't register as PE
            # activity (borrows one of the stream's psum buffers; freed ~13us)
            wsrc = aux.tile([P, 512], F16)
            nc.vector.memset(wsrc[:], 0.0)
            warm_ps = psp.tile([P, GW], F32, tag="ps")
            for _ in range(10):
                nc.tensor.matmul(out=warm_ps[:, 0:512], lhsT=wsrc[:, 0:P], rhs=wsrc[:], start=True, stop=True)

            # ---- stream bookkeeping (the chain never touches ACT, so the
            # stream and chain only share the DVE, where the chain comes
            # first and the per-block epilogues run after it finishes) ----
            rs = bigp.tile([P, NB, NG + 2], F32)  # extra slots: split group 0
            rsum = aux.tile([P, NB], F32)
            diag = aux.tile([P, NB], F32)
            mp2 = aux.tile([P, NB], F32)
            nc.vector.memset(rs[:, :, NG : NG + 2], 0.0)
            nc.scalar.activation(out=diag[:], in_=n16[:], func=EXPF, scale=1.0 / TEMP)
            order = [(b, g) for g in range(NG - 1) for b in range(NB - 1)]
            order += [(NB - 1, g) for g in range(NG - 1)]
            order += [(b, NG - 1) for b in range(NB)]

            def emit_group(b, g):
                ps = psp.tile([P, GW], F32, tag="ps")
                for s in range(GW // 512):
                    for h in range(2):
                        lh = lhsT7[h][:] if b == NB - 1 else ptT[h][:, b * P : (b + 1) * P]
                        nc.tensor.matmul(
                            out=ps[:, s * 512 : (s + 1) * 512],
                            lhsT=lh,
                            rhs=ptT[h][:, g * GW + s * 512 : g * GW + (s + 1) * 512],
                            start=(h == 0),
                            stop=(h == 1),
                        )
                scr = scrp.tile([P, GW], F16, tag="esc")
                if (b, g) == order[0]:
                    # split the very first group so exp starts on the first
                    # matmuls (pieces go to slots g, NG, NG+1)
                    for c0p, c1p, slot in ((0, 512, g), (512, 1024, NG), (1024, 2048, NG + 1)):
                        nc.scalar.activation(
                            out=scr[:, c0p:c1p],
                            in_=ps[:, c0p:c1p],
                            func=EXPF, scale=1.0 / TEMP,
                            accum_out=rs[:, b, slot : slot + 1],
                        )
                else:
                    nc.scalar.activation(
                        out=scr[:], in_=ps[:], func=EXPF, scale=1.0 / TEMP,
                        accum_out=rs[:, b, g : g + 1],
                    )
                if g == NG - 1:
                    # block b complete: per-row ln overlaps the remaining groups
                    nc.vector.tensor_reduce(
                        out=rsum[:, b : b + 1], in_=rs[:, b, :], axis=AX, op=ADD
                    )
                    nc.vector.tensor_sub(
                        rsum[:, b : b + 1], rsum[:, b : b + 1], diag[:, b : b + 1]
                    )
                    nc.scalar.activation(
                        out=mp2[:, b : b + 1], in_=rsum[:, b : b + 1],
                        func=LNF, scale=1.0 / (C - 1),
                    )
                    if b == NB - 2:
                        # ship the first 7 blocks early (warms the out ring);
                        # only block 7's column goes in the final tail DMA
                        nc.gpsimd.dma_start(out_d[:, 0 : NB - 1], mp2[:, 0 : NB - 1])

            # round 0: v1 = p + f  (one contiguous [P, NT*D] add, fp16 in f32 out)
            nc.vector.tensor_add(
                u[:, :, :], uf[:, :, :], fg[:, fo_list[0] : fo_list[0] + NT, :]
            )

            def finalize_norm(t):
                nc.vector.tensor_mul(sq[:], u[:, t, :], u[:, t, :])
                nc.vector.tensor_reduce(
                    out=n2[:, t : t + 1], in_=sq[:], axis=AX, op=ADD
                )

            def finalize_out(t):
                nc.vector.tensor_scalar_mul(u16[:, t, :], u[:, t, :], rinv[:, t : t + 1])

            def emit_transposes(t):
                # chain tile t -> ptT columns [KH + t*P, KH + (t+1)*P)
                for h in range(2):
                    nc.sync.dma_start_transpose(
                        ptT[h][:, KH + t * P : KH + (t + 1) * P],
                        u16[:, t, h * P : (h + 1) * P],
                    )
                if t == 0:  # tile 0 is also this core's own-row lhsT block 7
                    for h in range(2):
                        nc.sync.dma_start_transpose(
                            lhsT7[h][:], u16[:, t, h * P : (h + 1) * P]
                        )

            # rounds >=1 on tiles 0..RT-1 (zero feature rows = no-op lanes);
            # ||v|| = n2 * rsqrt(n2), all on the DVE
            for r in range(1, R):
                fo = fo_list[r]
                for t in range(RT):
                    finalize_norm(t)
                rsqrt_dve(0, RT)
                nc.vector.tensor_mul(nrm[:, 0:RT], n2[:, 0:RT], rinv[:, 0:RT])
                for t in range(RT):
                    nc.vector.tensor_scalar_mul(
                        fscl[:], fg[:, fo + t, :], nrm[:, t : t + 1]
                    )
                    nc.vector.tensor_add(u[:, t, :], u[:, t, :], fscl[:])

            # tile 0 fast path: block-7 lhsT is the earliest-needed transpose
            finalize_norm(0)
            rsqrt_dve(0, 1)
            finalize_out(0)
            emit_transposes(0)
            for t in range(1, NT):
                finalize_norm(t)
            rsqrt_dve(1, NT)
            for t in range(1, NT):
                finalize_out(t)
                emit_transposes(t)

            # ---- matmul + exp stream ----
            # order: chain-free work first; block 7 (own chain rows) and group 3
            # (chain columns) last, giving the chain ~40us of slack
            for b, g in order:
                emit_group(b, g)

            nc.gpsimd.dma_start(out_d[:, NB - 1 : NB], mp2[:, NB - 1 : NB])

    nc.compile()
    return nc


def _plan(labels_np):
    """Host-side class relabeling + chain layout.

    Returns per-core lane class lists (tile 0 = core's own rows), the per-core
    K-column rotation, and the chain round structure.
    """
    occ = {}
    for t, c in enumerate(labels_np):
        occ.setdefault(int(c), []).append(t)
    S = len(occ)
    assert S <= CPC
    multi = sorted((c for c, ts in occ.items() if len(ts) > 1),
                   key=lambda c: (-len(occ[c]), c))
    single = sorted(c for c, ts in occ.items() if len(ts) == 1)
    R = max(len(ts) for ts in occ.values())

    unused = sorted(set(range(C)) - set(occ))
    fill = unused[: CPC - S]          # untouched classes used as chain filler
    kcls = unused[CPC - S :]          # the KH K-column classes
    assert len(kcls) == KH

    # ownership: deal multi-occ then single-occ round-robin; sizes <= 128
    own = [[] for _ in range(NCORES)]
    for i, c in enumerate(multi):
        own[i % NCORES].append(c)
    for i, c in enumerate(single):
        own[(i + len(multi)) % NCORES].append(c)
    assert max(len(o) for o in own) <= P

    fill_iter = iter(fill)
    own_full = []
    for r0 in range(NCORES):
        o = list(own[r0])
        while len(o) < P:
            o.append(next(fill_iter))
        own_full.append(o)

    # per-core lane order: tile 0 = own classes; tiles 1.. = other cores' own
    # classes with ALL non-own multi-occ classes first (so rounds >=1 only
    # touch a fixed tile prefix)
    lanes = []
    maxnm = 0
    for r0 in range(NCORES):
        others = []
        for q in range(NCORES):
            if q != r0:
                others.extend(own_full[q])
        om = [c for c in others if len(occ.get(c, ())) > 1]
        os_ = [c for c in others if len(occ.get(c, ())) <= 1]
        maxnm = max(maxnm, len(om))
        lanes.append(own_full[r0] + om + os_)
    RT = 1 + (maxnm + P - 1) // P     # tiles touched by rounds >= 1
    fo_list = [0]
    off = NT
    for r in range(1, R):
        fo_list.append(off)
        off += RT
    NFT = off
    return lanes, occ, kcls, R, RT, NFT, fo_list


def prepare(features, prototypes, labels):
    """Host-side specialization: build the SPMD program and per-core inputs."""
    features = np.asarray(features, dtype=np.float32)
    prototypes = np.asarray(prototypes, dtype=np.float32)
    labels_np = np.asarray(labels).astype(np.int64)

    lanes, occ, kcls, R, RT, NFT, fo_list = _plan(labels_np)
    nc = build_program(NFT, R, RT, fo_list)

    kcls_arr = np.asarray(kcls, dtype=np.int64)
    proto16 = prototypes.astype(np.float16)

    in_maps = []
    for r0 in range(NCORES):
        lane = np.asarray(lanes[r0], dtype=np.int64)      # [CPC]
        # chain inputs: proto rows tile-major (tile t lane p = lane[t*128+p])
        rows = np.zeros((NT + NFT, P, D), dtype=np.float16)
        rows[0:NT] = proto16[lane].reshape(NT, P, D)
        f16 = features.astype(np.float16)
        f0 = np.zeros((CPC, D), dtype=np.float16)
        for L, c in enumerate(lane):
            ts = occ.get(int(c))
            if ts:
                f0[L] = f16[ts[0]]
        rows[NT : 2 * NT] = f0.reshape(NT, P, D)
        for r in range(1, R):
            fr = np.zeros((RT * P, D), dtype=np.float16)
            for L in range(RT * P):
                ts = occ.get(int(lane[L]))
                if ts and len(ts) > r:
                    fr[L] = f16[ts[r]]
            rows[NT + fo_list[r] : NT + fo_list[r] + RT] = fr.reshape(RT, P, D)
        ufg_host = np.ascontiguousarray(rows.transpose(1, 0, 2))  # [P, NT+NFT, D]

        # per-core-rotated K columns (own 896 first), transposed
        krot = np.roll(kcls_arr, -r0 * KC)
        ptTK = np.ascontiguousarray(proto16[krot].T.reshape(2, P, KH))

        # exact fp16 squared norms of own rows; block 7 (chain rows) -> 1.0
        n16 = np.ones((P, NB), dtype=np.float32)
        ownk16 = proto16[krot[: KC]].astype(np.float32)
        n16[:, : NB - 1] = (
            (ownk16 * ownk16).sum(axis=1).reshape(NB - 1, P).T
        )

        in_maps.append({"ufg": ufg_host, "ptTK": ptTK, "n16": n16})

    return nc, in_maps


def kernel(features, prototypes, labels):
    nc, in_maps = prepare(features, prototypes, labels)
    res = run_bass_kernel_spmd(nc, in_maps, list(range(NCORES)))
    total = sum(
        float(np.asarray(res.results[i]["partial"], dtype=np.float64).sum())
        for i in range(NCORES)
    )
    loss = (TEMP / BASE_TEMP) * (total / C)
    return np.asarray(loss, dtype=np.float32)


# revision 49
# speedup vs baseline: 1.0215x; 1.0215x over previous
"""Trainium2 Bass kernel for nn_DisLoss (prototype EMA + masked pairwise exp-sim loss).

Strategy (8 NeuronCores, SPMD, one compiled program + per-core data rotation):
  - The loss is invariant under any permutation of the class axis, and rows of
    the [C,C] logits matrix are independent.  The host therefore RE-LABELS the
    class space per core: the 1024 "chain lane" classes (the distinct labels
    plus untouched filler classes, whose zero-feature chains are identity maps)
    are ordered so that core r's tile 0 holds exactly its own 128 row-classes,
    and the 7168 untouched "K classes" are rotated per core so that its own 896
    K rows are columns 0..895 of its ptT copy (lhsT == ptT slice, no extra
    input).
  - The host pre-transposes the K-class prototype columns (ptTK, fp16), so the
    [rows x cols] matmul + exp stream starts as soon as the first column chunk
    lands (~12us), fully overlapping the EMA chain.  Chunk DMAs are serialized
    per HWDGE ring (the DGE round-robins packets of all outstanding descriptors,
    so without explicit ordering chunk 0 would land LAST).
  - The EMA chain (deferred-normalization form: v <- v + ||v||*f per round,
    normalize once at the end) runs replicated on the DVE (norms via mul +
    reduce) so the ACT engine stays dedicated to the exp stream.  Chain output
    tiles are xbar-DMA-transposed into the LAST 1024 columns of ptT and into
    the block-7 lhsT, which the matmul loop consumes last.
  - exp writes fp16 (max value exp(10) < fp16 max) and the per-row sums are
    accumulated by the DVE (tensor_scalar at 4x rate with accum_out), keeping
    the ~0.3us/group ACT accumulator-read off the critical engine.
  - Diagonal masking: row i's self-logit is ||row_i||^2/T; the host sends the
    exact fp16 squared norms and the device subtracts exp(10*n) per row before
    the final Ln.  Each core reduces its 1024 rows to one scalar; the host
    sums 8 scalars.
"""

import math
from contextlib import ExitStack

import numpy as np

import types as _pytypes

import bass_rust as _bass_rust
import concourse.bass as bass
import concourse.mybir as mybir
import concourse.tile as tile
from concourse import bacc
from concourse.bass_utils import run_bass_kernel_spmd
from concourse.hw_specs import get_activation_tables
from concourse.tile_rust import add_dep_helper

ACT_SET = "natural_log_exp_and_others"  # contains Exp + Ln


def _pin_act_tables(nc):
    """Force all activations onto one table set (avoids ~2.7us set reloads)."""

    def patched(self):
        has_act = any(
            isinstance(i, mybir.InstActivation)
            for b in self.main_func.blocks
            for i in b.instructions
        )
        if not has_act:
            return
        tables = [
            (name, fns if name == ACT_SET else type(fns)())
            for name, fns in get_activation_tables(self.m.arch).items()
        ]
        _bass_rust.insert_act_table_loads(self, tables)

    nc.insert_act_table_loads = _pytypes.MethodType(patched, nc)


P = 128
C = 8192
D = 256
B = 1024
NCORES = 8
CPC = C // NCORES          # row classes per core (1024)
NT = CPC // P              # chain lane tiles (8); chain lanes == CPC
KC = CPC - P               # own K rows per core (896) == ptT cols 0..895
KH = C - CPC               # K-class columns (7168)
NB = CPC // P              # own row blocks (8)
GW = 2048                  # psum group width (4 banks, double-buffered)
NG = C // GW               # col groups (4)
TEMP = 0.1
BASE_TEMP = 0.1
EXP10 = math.exp(1.0 / TEMP)

F32 = mybir.dt.float32
F16 = mybir.dt.float16
I32 = mybir.dt.int32
AX = mybir.AxisListType.X
ADD = mybir.AluOpType.add
SUB = mybir.AluOpType.subtract
MULT = mybir.AluOpType.mult
EXPF = mybir.ActivationFunctionType.Exp
LNF = mybir.ActivationFunctionType.Ln


def _ins(x):
    return getattr(x, "ins", x)


def _chain_dma(prev, cur):
    if prev is not None:
        add_dep_helper(_ins(cur), _ins(prev), sync=True, reason="serialize ring")
    return cur


def build_program(NFT, R, RT, fo_list):
    """One SPMD Bass program; all shape-relevant values are core-invariant."""
    nc = bacc.Bacc("TRN2", target_bir_lowering=False, debug=False, num_devices=NCORES)
    _pin_act_tables(nc)

    # ufg: [chain-lane proto rows (NT tiles); per-round feature rows (NFT tiles)]
    ufg_d = nc.declare_dram_parameter("ufg", [P, NT + NFT, D], F16, isOutput=False)
    # per-core-rotated K columns, pre-transposed: ptTK[h][d, j] = proto16[krot[j], h*128+d]
    ptTK_d = nc.declare_dram_parameter("ptTK", [2, P, KH], F16, isOutput=False)
    # exact fp16 squared norms of own rows (block 7 slot = 1.0)
    n16_d = nc.declare_dram_parameter("n16", [P, NB], F32, isOutput=False)
    # per-row ln((rowsum - exp(10*n))/(C-1)); host sums across rows and cores
    out_d = nc.declare_dram_parameter("partial", [P, NB], F32, isOutput=True)

    with tile.TileContext(nc) as tc:
        with ExitStack() as ctx:
            aux = ctx.enter_context(tc.tile_pool(name="aux", bufs=1))
            chainp = ctx.enter_context(tc.tile_pool(name="chain", bufs=1))
            psp = ctx.enter_context(tc.tile_pool(name="ps", bufs=2, space="PSUM"))
            bigp = ctx.enter_context(tc.tile_pool(name="big", bufs=1))
            scrp = ctx.enter_context(tc.tile_pool(name="scr", bufs=2))

            ones_sb = aux.tile([P, 1], F32)
            nc.vector.memset(ones_sb[:], 1.0)

            # ---- input DMAs ----
            # Ring bandwidths (measured): gpsimd ~200 GB/s, scalar ~95, sync
            # ~48.  The DGE round-robins outstanding descriptors, so the
            # gpsimd ring is explicitly chained in consumption order.  The
            # scalar ring only gets UNCHAINED kicks (a chained kick's wait
            # would block the ACT engine mid-stream).
            ptT = [bigp.tile([P, C], F16, name=f"ptT{h}") for h in range(2)]
            n16 = aux.tile([P, NB], F32)
            nc.sync.dma_start(n16[:], n16_d[:])
            ufg = chainp.tile([P, NT + NFT, D], F16)
            HG = GW // 2
            # scalar ring kicks earliest (short ACT preamble), fire-and-forget:
            # the first 1024-col half plus col group 2
            nc.scalar.dma_start(ptT[0][:, 0:HG], ptTK_d[0, :, 0:HG])
            nc.scalar.dma_start(ptT[1][:, 0:HG], ptTK_d[1, :, 0:HG])
            nc.scalar.dma_start(ptT[0][:, 2 * GW : 3 * GW], ptTK_d[0, :, 2 * GW : 3 * GW])
            nc.scalar.dma_start(ptT[1][:, 2 * GW : 3 * GW], ptTK_d[1, :, 2 * GW : 3 * GW])
            # force the (single) activation table set to load while DMAs run
            # (after the scalar-ring kicks -- the ACT queue is strict FIFO)
            dummy = aux.tile([1, 1], F32)
            nc.scalar.activation(out=dummy[:], in_=ones_sb[0:1, 0:1], func=LNF)
            # gpsimd ring (fast), FIFO order = consumption order: rest of group
            # 0, chain inputs (longest downstream path: chain -> 18 slow
            # sync-ring transposes), group 1, group 3's K part
            nc.gpsimd.dma_start(ptT[0][:, HG:GW], ptTK_d[0, :, HG:GW])
            nc.gpsimd.dma_start(ptT[1][:, HG:GW], ptTK_d[1, :, HG:GW])
            nc.gpsimd.dma_start(ufg[:, :, :], ufg_d[:, :, :])
            nc.gpsimd.dma_start(ptT[0][:, GW : 2 * GW], ptTK_d[0, :, GW : 2 * GW])
            nc.gpsimd.dma_start(ptT[1][:, GW : 2 * GW], ptTK_d[1, :, GW : 2 * GW])
            nc.gpsimd.dma_start(ptT[0][:, 3 * GW : KH], ptTK_d[0, :, 3 * GW : KH])
            nc.gpsimd.dma_start(ptT[1][:, 3 * GW : KH], ptTK_d[1, :, 3 * GW : KH])

            # ---- EMA chain, DVE-resident (ACT only for tiny Ln/Exp on norms) ----
            uf = ufg[:, 0:NT, :]
            fg = ufg[:, NT:, :]
            u = chainp.tile([P, NT, D], F32)
            sq = chainp.tile([P, D], F32)
            n2 = chainp.tile([P, NT], F32)
            rinv = chainp.tile([P, NT], F32)
            tmp = chainp.tile([P, NT], F32)
            nrm = chainp.tile([P, NT], F32)
            magic = chainp.tile([P, NT], I32)
            nc.vector.memset(magic[:], 0x5F3759DF)
            fscl = chainp.tile([P, D], F32)
            u16 = chainp.tile([P, NT, D], F16)
            lhsT7 = [aux.tile([P, P], F16, name=f"lhsT7{h}") for h in range(2)]

            def rsqrt_dve(lo, hi):
                """rinv[:, lo:hi] = 1/sqrt(n2[:, lo:hi]) entirely on the DVE
                (fast-inverse-sqrt bit trick + 2 Newton iterations, rel err
                ~4e-6) so the chain never touches the ACT engine."""
                a = lambda t: t[:, lo:hi]
                nc.vector.tensor_scalar(
                    out=a(tmp).bitcast(I32), in0=a(n2).bitcast(I32),
                    scalar1=1, scalar2=None, op0=mybir.AluOpType.logical_shift_right,
                )
                nc.vector.tensor_sub(a(rinv).bitcast(I32), a(magic), a(tmp).bitcast(I32))
                for _ in range(2):
                    nc.vector.tensor_mul(a(tmp), a(rinv), a(rinv))
                    nc.vector.tensor_mul(a(tmp), a(tmp), a(n2))
                    nc.vector.tensor_scalar(
                        out=a(tmp), in0=a(tmp),
                        scalar1=-0.5, scalar2=1.5, op0=MULT, op1=ADD,
                    )
                    nc.vector.tensor_mul(a(rinv), a(rinv), a(tmp))

            # warm the PE's HAM clock gate while the first column chunks land:
            # must be FAT matmuls (N=512) -- tiny ones don't register as PE
            # activity (borrows one of the stream's psum buffers; freed ~13us)
            wsrc = aux.tile([P, 512], F16)
            nc.vector.memset(wsrc[:], 0.0)
            warm_ps = psp.tile([P, GW], F32, tag="ps")
            for _ in range(16):
                nc.tensor.matmul(out=warm_ps[:, 0:512], lhsT=wsrc[:, 0:P], rhs=wsrc[:], start=True, stop=True)

            # ---- stream bookkeeping (the chain never touches ACT, so the
            # stream and chain only share the DVE, where the chain comes
            # first and the per-block epilogues run after it finishes) ----
            rs = bigp.tile([P, NB, NG + 2], F32)  # extra slots: split group 0
            rsum = aux.tile([P, NB], F32)
            diag = aux.tile([P, NB], F32)
            mp2 = aux.tile([P, NB], F32)
            nc.vector.memset(rs[:, :, NG : NG + 2], 0.0)
            nc.scalar.activation(out=diag[:], in_=n16[:], func=EXPF, scale=1.0 / TEMP)
            order = [(b, g) for g in range(NG - 1) for b in range(NB - 1)]
            order += [(NB - 1, g) for g in range(NG - 1)]
            order += [(b, NG - 1) for b in range(NB)]

            def emit_group(b, g):
                ps = psp.tile([P, GW], F32, tag="ps")
                for s in range(GW // 512):
                    for h in range(2):
                        lh = lhsT7[h][:] if b == NB - 1 else ptT[h][:, b * P : (b + 1) * P]
                        nc.tensor.matmul(
                            out=ps[:, s * 512 : (s + 1) * 512],
                            lhsT=lh,
                            rhs=ptT[h][:, g * GW + s * 512 : g * GW + (s + 1) * 512],
                            start=(h == 0),
                            stop=(h == 1),
                        )
                scr = scrp.tile([P, GW], F16, tag="esc")
                if (b, g) == order[0]:
                    # split the very first group so exp starts on the first
                    # half-chunk of column DMA (halves go to slots g and NG)
                    for c0p, c1p, slot in ((0, 1024, g), (1024, 2048, NG)):
                        nc.scalar.activation(
                            out=scr[:, c0p:c1p],
                            in_=ps[:, c0p:c1p],
                            func=EXPF, scale=1.0 / TEMP,
                            accum_out=rs[:, b, slot : slot + 1],
                        )
                else:
                    nc.scalar.activation(
                        out=scr[:], in_=ps[:], func=EXPF, scale=1.0 / TEMP,
                        accum_out=rs[:, b, g : g + 1],
                    )
                if g == NG - 1:
                    # block b complete: per-row ln overlaps the remaining groups
                    nc.vector.tensor_reduce(
                        out=rsum[:, b : b + 1], in_=rs[:, b, :], axis=AX, op=ADD
                    )
                    nc.vector.tensor_sub(
                        rsum[:, b : b + 1], rsum[:, b : b + 1], diag[:, b : b + 1]
                    )
                    nc.scalar.activation(
                        out=mp2[:, b : b + 1], in_=rsum[:, b : b + 1],
                        func=LNF, scale=1.0 / (C - 1),
                    )
                    if b == NB - 2:
                        # ship the first 7 blocks early (warms the out ring);
                        # only block 7's column goes in the final tail DMA
                        nc.gpsimd.dma_start(out_d[:, 0 : NB - 1], mp2[:, 0 : NB - 1])

            # round 0: v1 = p + f  (one contiguous [P, NT*D] add, fp16 in f32 out)
            nc.vector.tensor_add(
                u[:, :, :], uf[:, :, :], fg[:, fo_list[0] : fo_list[0] + NT, :]
            )

            def finalize_norm(t):
                nc.vector.tensor_mul(sq[:], u[:, t, :], u[:, t, :])
                nc.vector.tensor_reduce(
                    out=n2[:, t : t + 1], in_=sq[:], axis=AX, op=ADD
                )

            def finalize_out(t):
                nc.vector.tensor_scalar_mul(u16[:, t, :], u[:, t, :], rinv[:, t : t + 1])

            def emit_transposes(t):
                # chain tile t -> ptT columns [KH + t*P, KH + (t+1)*P)
                for h in range(2):
                    nc.sync.dma_start_transpose(
                        ptT[h][:, KH + t * P : KH + (t + 1) * P],
                        u16[:, t, h * P : (h + 1) * P],
                    )
                if t == 0:  # tile 0 is also this core's own-row lhsT block 7
                    for h in range(2):
                        nc.sync.dma_start_transpose(
                            lhsT7[h][:], u16[:, t, h * P : (h + 1) * P]
                        )

            # rounds >=1 on tiles 0..RT-1 (zero feature rows = no-op lanes);
            # ||v|| = n2 * rsqrt(n2), all on the DVE
            for r in range(1, R):
                fo = fo_list[r]
                for t in range(RT):
                    finalize_norm(t)
                rsqrt_dve(0, RT)
                nc.vector.tensor_mul(nrm[:, 0:RT], n2[:, 0:RT], rinv[:, 0:RT])
                for t in range(RT):
                    nc.vector.tensor_scalar_mul(
                        fscl[:], fg[:, fo + t, :], nrm[:, t : t + 1]
                    )
                    nc.vector.tensor_add(u[:, t, :], u[:, t, :], fscl[:])

            # tile 0 fast path: block-7 lhsT is the earliest-needed transpose
            finalize_norm(0)
            rsqrt_dve(0, 1)
            finalize_out(0)
            emit_transposes(0)
            for t in range(1, NT):
                finalize_norm(t)
            rsqrt_dve(1, NT)
            for t in range(1, NT):
                finalize_out(t)
                emit_transposes(t)

            # ---- matmul + exp stream ----
            # order: chain-free work first; block 7 (own chain rows) and group 3
            # (chain columns) last, giving the chain ~40us of slack
            for b, g in order:
                emit_group(b, g)

            nc.gpsimd.dma_start(out_d[:, NB - 1 : NB], mp2[:, NB - 1 : NB])

    nc.compile()
    return nc


def _plan(labels_np):
    """Host-side class relabeling + chain layout.

    Returns per-core lane class lists (tile 0 = core's own rows), the per-core
    K-column rotation, and the chain round structure.
    """
    occ = {}
    for t, c in enumerate(labels_np):
        occ.setdefault(int(c), []).append(t)
    S = len(occ)
    assert S <= CPC
    multi = sorted((c for c, ts in occ.items() if len(ts) > 1),
                   key=lambda c: (-len(occ[c]), c))
    single = sorted(c for c, ts in occ.items() if len(ts) == 1)
    R = max(len(ts) for ts in occ.values())

    unused = sorted(set(range(C)) - set(occ))
    fill = unused[: CPC - S]          # untouched classes used as chain filler
    kcls = unused[CPC - S :]          # the KH K-column classes
    assert len(kcls) == KH

    # ownership: deal multi-occ then single-occ round-robin; sizes <= 128
    own = [[] for _ in range(NCORES)]
    for i, c in enumerate(multi):
        own[i % NCORES].append(c)
    for i, c in enumerate(single):
        own[(i + len(multi)) % NCORES].append(c)
    assert max(len(o) for o in own) <= P

    fill_iter = iter(fill)
    own_full = []
    for r0 in range(NCORES):
        o = list(own[r0])
        while len(o) < P:
            o.append(next(fill_iter))
        own_full.append(o)

    # per-core lane order: tile 0 = own classes; tiles 1.. = other cores' own
    # classes with ALL non-own multi-occ classes first (so rounds >=1 only
    # touch a fixed tile prefix)
    lanes = []
    maxnm = 0
    for r0 in range(NCORES):
        others = []
        for q in range(NCORES):
            if q != r0:
                others.extend(own_full[q])
        om = [c for c in others if len(occ.get(c, ())) > 1]
        os_ = [c for c in others if len(occ.get(c, ())) <= 1]
        maxnm = max(maxnm, len(om))
        lanes.append(own_full[r0] + om + os_)
    RT = 1 + (maxnm + P - 1) // P     # tiles touched by rounds >= 1
    fo_list = [0]
    off = NT
    for r in range(1, R):
        fo_list.append(off)
        off += RT
    NFT = off
    return lanes, occ, kcls, R, RT, NFT, fo_list


def prepare(features, prototypes, labels):
    """Host-side specialization: build the SPMD program and per-core inputs."""
    features = np.asarray(features, dtype=np.float32)
    prototypes = np.asarray(prototypes, dtype=np.float32)
    labels_np = np.asarray(labels).astype(np.int64)

    lanes, occ, kcls, R, RT, NFT, fo_list = _plan(labels_np)
    nc = build_program(NFT, R, RT, fo_list)

    kcls_arr = np.asarray(kcls, dtype=np.int64)
    proto16 = prototypes.astype(np.float16)

    in_maps = []
    for r0 in range(NCORES):
        lane = np.asarray(lanes[r0], dtype=np.int64)      # [CPC]
        # chain inputs: proto rows tile-major (tile t lane p = lane[t*128+p])
        rows = np.zeros((NT + NFT, P, D), dtype=np.float16)
        rows[0:NT] = proto16[lane].reshape(NT, P, D)
        f16 = features.astype(np.float16)
        f0 = np.zeros((CPC, D), dtype=np.float16)
        for L, c in enumerate(lane):
            ts = occ.get(int(c))
            if ts:
                f0[L] = f16[ts[0]]
        rows[NT : 2 * NT] = f0.reshape(NT, P, D)
        for r in range(1, R):
            fr = np.zeros((RT * P, D), dtype=np.float16)
            for L in range(RT * P):
                ts = occ.get(int(lane[L]))
                if ts and len(ts) > r:
                    fr[L] = f16[ts[r]]
            rows[NT + fo_list[r] : NT + fo_list[r] + RT] = fr.reshape(RT, P, D)
        ufg_host = np.ascontiguousarray(rows.transpose(1, 0, 2))  # [P, NT+NFT, D]

        # per-core-rotated K columns (own 896 first), transposed
        krot = np.roll(kcls_arr, -r0 * KC)
        ptTK = np.ascontiguousarray(proto16[krot].T.reshape(2, P, KH))

        # exact fp16 squared norms of own rows; block 7 (chain rows) -> 1.0
        n16 = np.ones((P, NB), dtype=np.float32)
        ownk16 = proto16[krot[: KC]].astype(np.float32)
        n16[:, : NB - 1] = (
            (ownk16 * ownk16).sum(axis=1).reshape(NB - 1, P).T
        )

        in_maps.append({"ufg": ufg_host, "ptTK": ptTK, "n16": n16})

    return nc, in_maps


def kernel(features, prototypes, labels):
    nc, in_maps = prepare(features, prototypes, labels)
    res = run_bass_kernel_spmd(nc, in_maps, list(range(NCORES)))
    total = sum(
        float(np.asarray(res.results[i]["partial"], dtype=np.float64).sum())
        for i in range(NCORES)
    )
    loss = (TEMP / BASE_TEMP) * (total / C)
    return np.asarray(loss, dtype=np.float32)


# revision 51
# speedup vs baseline: 1.0332x; 1.0114x over previous
"""Trainium2 Bass kernel for nn_DisLoss (prototype EMA + masked pairwise exp-sim loss).

Strategy (8 NeuronCores, SPMD, one compiled program + per-core data rotation):
  - The loss is invariant under any permutation of the class axis, and rows of
    the [C,C] logits matrix are independent.  The host therefore RE-LABELS the
    class space per core: the 1024 "chain lane" classes (the distinct labels
    plus untouched filler classes, whose zero-feature chains are identity maps)
    are ordered so that core r's tile 0 holds exactly its own 128 row-classes,
    and the 7168 untouched "K classes" are rotated per core so that its own 896
    K rows are columns 0..895 of its ptT copy (lhsT == ptT slice, no extra
    input).
  - The host pre-transposes the K-class prototype columns (ptTK, fp16), so the
    [rows x cols] matmul + exp stream starts as soon as the first column chunk
    lands (~12us), fully overlapping the EMA chain.  Chunk DMAs are serialized
    per HWDGE ring (the DGE round-robins packets of all outstanding descriptors,
    so without explicit ordering chunk 0 would land LAST).
  - The EMA chain (deferred-normalization form: v <- v + ||v||*f per round,
    normalize once at the end) runs replicated on the DVE (norms via mul +
    reduce) so the ACT engine stays dedicated to the exp stream.  Chain output
    tiles are xbar-DMA-transposed into the LAST 1024 columns of ptT and into
    the block-7 lhsT, which the matmul loop consumes last.
  - exp writes fp16 (max value exp(10) < fp16 max) and the per-row sums are
    accumulated by the DVE (tensor_scalar at 4x rate with accum_out), keeping
    the ~0.3us/group ACT accumulator-read off the critical engine.
  - Diagonal masking: row i's self-logit is ||row_i||^2/T; the host sends the
    exact fp16 squared norms and the device subtracts exp(10*n) per row before
    the final Ln.  Each core reduces its 1024 rows to one scalar; the host
    sums 8 scalars.
"""

import math
from contextlib import ExitStack

import numpy as np

import types as _pytypes

import bass_rust as _bass_rust
import concourse.bass as bass
import concourse.mybir as mybir
import concourse.tile as tile
from concourse import bacc
from concourse.bass_utils import run_bass_kernel_spmd
from concourse.hw_specs import get_activation_tables
from concourse.tile_rust import add_dep_helper

ACT_SET = "natural_log_exp_and_others"  # contains Exp + Ln


def _pin_act_tables(nc):
    """Force all activations onto one table set (avoids ~2.7us set reloads)."""

    def patched(self):
        has_act = any(
            isinstance(i, mybir.InstActivation)
            for b in self.main_func.blocks
            for i in b.instructions
        )
        if not has_act:
            return
        tables = [
            (name, fns if name == ACT_SET else type(fns)())
            for name, fns in get_activation_tables(self.m.arch).items()
        ]
        _bass_rust.insert_act_table_loads(self, tables)

    nc.insert_act_table_loads = _pytypes.MethodType(patched, nc)


P = 128
C = 8192
D = 256
B = 1024
NCORES = 8
CPC = C // NCORES          # row classes per core (1024)
NT = CPC // P              # chain lane tiles (8); chain lanes == CPC
KC = CPC - P               # own K rows per core (896) == ptT cols 0..895
KH = C - CPC               # K-class columns (7168)
NB = CPC // P              # own row blocks (8)
GW = 2048                  # psum group width (4 banks, double-buffered)
NG = C // GW               # col groups (4)
TEMP = 0.1
BASE_TEMP = 0.1
EXP10 = math.exp(1.0 / TEMP)

F32 = mybir.dt.float32
F16 = mybir.dt.float16
I32 = mybir.dt.int32
AX = mybir.AxisListType.X
ADD = mybir.AluOpType.add
SUB = mybir.AluOpType.subtract
MULT = mybir.AluOpType.mult
EXPF = mybir.ActivationFunctionType.Exp
LNF = mybir.ActivationFunctionType.Ln


def _ins(x):
    return getattr(x, "ins", x)


def _chain_dma(prev, cur):
    if prev is not None:
        add_dep_helper(_ins(cur), _ins(prev), sync=True, reason="serialize ring")
    return cur


def build_program(NFT, R, RT, fo_list):
    """One SPMD Bass program; all shape-relevant values are core-invariant."""
    nc = bacc.Bacc("TRN2", target_bir_lowering=False, debug=False, num_devices=NCORES)
    _pin_act_tables(nc)

    # ufg: [chain-lane proto rows (NT tiles); per-round feature rows (NFT tiles)]
    ufg_d = nc.declare_dram_parameter("ufg", [P, NT + NFT, D], F16, isOutput=False)
    # per-core-rotated K columns, pre-transposed: ptTK[h][d, j] = proto16[krot[j], h*128+d]
    ptTK_d = nc.declare_dram_parameter("ptTK", [2, P, KH], F16, isOutput=False)
    # exact fp16 squared norms of own rows (block 7 slot = 1.0)
    n16_d = nc.declare_dram_parameter("n16", [P, NB], F32, isOutput=False)
    # per-row ln((rowsum - exp(10*n))/(C-1)); host sums across rows and cores
    out_d = nc.declare_dram_parameter("partial", [P, NB], F32, isOutput=True)

    with tile.TileContext(nc) as tc:
        with ExitStack() as ctx:
            aux = ctx.enter_context(tc.tile_pool(name="aux", bufs=1))
            chainp = ctx.enter_context(tc.tile_pool(name="chain", bufs=1))
            psp = ctx.enter_context(tc.tile_pool(name="ps", bufs=2, space="PSUM"))
            bigp = ctx.enter_context(tc.tile_pool(name="big", bufs=1))
            scrp = ctx.enter_context(tc.tile_pool(name="scr", bufs=2))

            ones_sb = aux.tile([P, 1], F32)
            nc.vector.memset(ones_sb[:], 1.0)

            # ---- input DMAs ----
            # Ring bandwidths (measured): gpsimd ~200 GB/s, scalar ~95, sync
            # ~48.  The DGE round-robins outstanding descriptors, so the
            # gpsimd ring is explicitly chained in consumption order.  The
            # scalar ring only gets UNCHAINED kicks (a chained kick's wait
            # would block the ACT engine mid-stream).
            ptT = [bigp.tile([P, C], F16, name=f"ptT{h}") for h in range(2)]
            n16 = aux.tile([P, NB], F32)
            nc.sync.dma_start(n16[:], n16_d[:])
            ufg = chainp.tile([P, NT + NFT, D], F16)
            HG = GW // 2
            # scalar ring kicks earliest (short ACT preamble), fire-and-forget:
            # the first 1024-col half plus col group 2
            nc.scalar.dma_start(ptT[0][:, 0:HG], ptTK_d[0, :, 0:HG])
            nc.scalar.dma_start(ptT[1][:, 0:HG], ptTK_d[1, :, 0:HG])
            nc.scalar.dma_start(ptT[0][:, 2 * GW : 3 * GW], ptTK_d[0, :, 2 * GW : 3 * GW])
            nc.scalar.dma_start(ptT[1][:, 2 * GW : 3 * GW], ptTK_d[1, :, 2 * GW : 3 * GW])
            # force the (single) activation table set to load while DMAs run
            # (after the scalar-ring kicks -- the ACT queue is strict FIFO)
            dummy = aux.tile([1, 1], F32)
            nc.scalar.activation(out=dummy[:], in_=ones_sb[0:1, 0:1], func=LNF)
            # gpsimd ring (fast), FIFO order = consumption order: rest of group
            # 0, chain inputs (longest downstream path: chain -> 18 slow
            # sync-ring transposes), group 1, group 3's K part
            nc.gpsimd.dma_start(ptT[0][:, HG:GW], ptTK_d[0, :, HG:GW])
            nc.gpsimd.dma_start(ptT[1][:, HG:GW], ptTK_d[1, :, HG:GW])
            nc.gpsimd.dma_start(ufg[:, :, :], ufg_d[:, :, :])
            nc.gpsimd.dma_start(ptT[0][:, GW : 2 * GW], ptTK_d[0, :, GW : 2 * GW])
            nc.gpsimd.dma_start(ptT[1][:, GW : 2 * GW], ptTK_d[1, :, GW : 2 * GW])
            nc.gpsimd.dma_start(ptT[0][:, 3 * GW : KH], ptTK_d[0, :, 3 * GW : KH])
            nc.gpsimd.dma_start(ptT[1][:, 3 * GW : KH], ptTK_d[1, :, 3 * GW : KH])

            # ---- EMA chain, DVE-resident (ACT only for tiny Ln/Exp on norms) ----
            uf = ufg[:, 0:NT, :]
            fg = ufg[:, NT:, :]
            u = chainp.tile([P, NT, D], F32)
            sq = chainp.tile([P, D], F32)
            n2 = chainp.tile([P, NT], F32)
            rinv = chainp.tile([P, NT], F32)
            tmp = chainp.tile([P, NT], F32)
            nrm = chainp.tile([P, NT], F32)
            magic = chainp.tile([P, NT], I32)
            nc.vector.memset(magic[:], 0x5F3759DF)
            fscl = chainp.tile([P, D], F32)
            u16 = chainp.tile([P, NT, D], F16)
            lhsT7 = [aux.tile([P, P], F16, name=f"lhsT7{h}") for h in range(2)]

            def rsqrt_dve(lo, hi):
                """rinv[:, lo:hi] = 1/sqrt(n2[:, lo:hi]) entirely on the DVE
                (fast-inverse-sqrt bit trick + 2 Newton iterations, rel err
                ~4e-6) so the chain never touches the ACT engine."""
                a = lambda t: t[:, lo:hi]
                nc.vector.tensor_scalar(
                    out=a(tmp).bitcast(I32), in0=a(n2).bitcast(I32),
                    scalar1=1, scalar2=None, op0=mybir.AluOpType.logical_shift_right,
                )
                nc.vector.tensor_sub(a(rinv).bitcast(I32), a(magic), a(tmp).bitcast(I32))
                for _ in range(2):
                    nc.vector.tensor_mul(a(tmp), a(rinv), a(rinv))
                    nc.vector.tensor_mul(a(tmp), a(tmp), a(n2))
                    nc.vector.tensor_scalar(
                        out=a(tmp), in0=a(tmp),
                        scalar1=-0.5, scalar2=1.5, op0=MULT, op1=ADD,
                    )
                    nc.vector.tensor_mul(a(rinv), a(rinv), a(tmp))

            # warm the PE's HAM clock gate while the first column chunks land:
            # must be FAT matmuls (N=512) -- tiny ones don't register as PE
            # activity (borrows one of the stream's psum buffers; freed ~13us)
            wsrc = aux.tile([P, 512], F16)
            nc.vector.memset(wsrc[:], 0.0)
            warm_ps = psp.tile([P, GW], F32, tag="ps")
            for _ in range(16):
                nc.tensor.matmul(out=warm_ps[:, 0:512], lhsT=wsrc[:, 0:P], rhs=wsrc[:], start=True, stop=True)

            # ---- stream bookkeeping (the chain never touches ACT, so the
            # stream and chain only share the DVE, where the chain comes
            # first and the per-block epilogues run after it finishes) ----
            rs = bigp.tile([P, NB, NG + 2], F32)  # extra slots: split group 0
            rsum = aux.tile([P, NB], F32)
            diag = aux.tile([P, NB], F32)
            mp2 = aux.tile([P, NB], F32)
            nc.vector.memset(rs[:, :, NG : NG + 2], 0.0)
            nc.scalar.activation(out=diag[:], in_=n16[:], func=EXPF, scale=1.0 / TEMP)
            order = [(b, g) for g in range(NG - 1) for b in range(NB - 1)]
            order += [(NB - 1, g) for g in range(NG - 1)]
            order += [(b, NG - 1) for b in range(NB)]

            def emit_group(b, g):
                ps = psp.tile([P, GW], F32, tag="ps")
                MW = 512  # one PSUM bank per matmul
                for s in range(GW // MW):
                    for h in range(2):
                        lh = lhsT7[h][:] if b == NB - 1 else ptT[h][:, b * P : (b + 1) * P]
                        nc.tensor.matmul(
                            out=ps[:, s * MW : (s + 1) * MW],
                            lhsT=lh,
                            rhs=ptT[h][:, g * GW + s * MW : g * GW + (s + 1) * MW],
                            start=(h == 0),
                            stop=(h == 1),
                        )
                scr = scrp.tile([P, GW], F16, tag="esc")
                if (b, g) == order[0]:
                    # split the very first group so exp starts on the first
                    # half-chunk of column DMA (halves go to slots g and NG)
                    for c0p, c1p, slot in ((0, 1024, g), (1024, 2048, NG)):
                        nc.scalar.activation(
                            out=scr[:, c0p:c1p],
                            in_=ps[:, c0p:c1p],
                            func=EXPF, scale=1.0 / TEMP,
                            accum_out=rs[:, b, slot : slot + 1],
                        )
                else:
                    nc.scalar.activation(
                        out=scr[:], in_=ps[:], func=EXPF, scale=1.0 / TEMP,
                        accum_out=rs[:, b, g : g + 1],
                    )
                if g == NG - 1:
                    # block b complete: per-row ln overlaps the remaining groups
                    nc.vector.tensor_reduce(
                        out=rsum[:, b : b + 1], in_=rs[:, b, :], axis=AX, op=ADD
                    )
                    nc.vector.tensor_sub(
                        rsum[:, b : b + 1], rsum[:, b : b + 1], diag[:, b : b + 1]
                    )
                    nc.scalar.activation(
                        out=mp2[:, b : b + 1], in_=rsum[:, b : b + 1],
                        func=LNF, scale=1.0 / (C - 1),
                    )
                    if b == NB - 2:
                        # ship the first 7 blocks early (warms the out ring);
                        # only block 7's column goes in the final tail DMA
                        nc.gpsimd.dma_start(out_d[:, 0 : NB - 1], mp2[:, 0 : NB - 1])

            # round 0: v1 = p + f  (one contiguous [P, NT*D] add, fp16 in f32 out)
            nc.vector.tensor_add(
                u[:, :, :], uf[:, :, :], fg[:, fo_list[0] : fo_list[0] + NT, :]
            )

            def finalize_norm(t):
                nc.vector.tensor_mul(sq[:], u[:, t, :], u[:, t, :])
                nc.vector.tensor_reduce(
                    out=n2[:, t : t + 1], in_=sq[:], axis=AX, op=ADD
                )

            def finalize_out(t):
                nc.vector.tensor_scalar_mul(u16[:, t, :], u[:, t, :], rinv[:, t : t + 1])

            def emit_transposes(t):
                # chain tile t -> ptT columns [KH + t*P, KH + (t+1)*P)
                for h in range(2):
                    nc.sync.dma_start_transpose(
                        ptT[h][:, KH + t * P : KH + (t + 1) * P],
                        u16[:, t, h * P : (h + 1) * P],
                    )
                if t == 0:  # tile 0 is also this core's own-row lhsT block 7
                    for h in range(2):
                        nc.sync.dma_start_transpose(
                            lhsT7[h][:], u16[:, t, h * P : (h + 1) * P]
                        )

            # rounds >=1 on tiles 0..RT-1 (zero feature rows = no-op lanes);
            # ||v|| = n2 * rsqrt(n2), all on the DVE
            for r in range(1, R):
                fo = fo_list[r]
                for t in range(RT):
                    finalize_norm(t)
                rsqrt_dve(0, RT)
                nc.vector.tensor_mul(nrm[:, 0:RT], n2[:, 0:RT], rinv[:, 0:RT])
                for t in range(RT):
                    nc.vector.tensor_scalar_mul(
                        fscl[:], fg[:, fo + t, :], nrm[:, t : t + 1]
                    )
                    nc.vector.tensor_add(u[:, t, :], u[:, t, :], fscl[:])

            # tile 0 fast path: block-7 lhsT is the earliest-needed transpose
            finalize_norm(0)
            rsqrt_dve(0, 1)
            finalize_out(0)
            emit_transposes(0)
            for t in range(1, NT):
                finalize_norm(t)
            rsqrt_dve(1, NT)
            for t in range(1, NT):
                finalize_out(t)
                emit_transposes(t)

            # ---- matmul + exp stream ----
            # order: chain-free work first; block 7 (own chain rows) and group 3
            # (chain columns) last, giving the chain ~40us of slack
            for b, g in order:
                emit_group(b, g)

            nc.gpsimd.dma_start(out_d[:, NB - 1 : NB], mp2[:, NB - 1 : NB])

    nc.compile()
    return nc


def _plan(labels_np):
    """Host-side class relabeling + chain layout.

    Returns per-core lane class lists (tile 0 = core's own rows), the per-core
    K-column rotation, and the chain round structure.
    """
    occ = {}
    for t, c in enumerate(labels_np):
        occ.setdefault(int(c), []).append(t)
    S = len(occ)
    assert S <= CPC
    multi = sorted((c for c, ts in occ.items() if len(ts) > 1),
                   key=lambda c: (-len(occ[c]), c))
    single = sorted(c for c, ts in occ.items() if len(ts) == 1)
    R = max(len(ts) for ts in occ.values())

    unused = sorted(set(range(C)) - set(occ))
    fill = unused[: CPC - S]          # untouched classes used as chain filler
    kcls = unused[CPC - S :]          # the KH K-column classes
    assert len(kcls) == KH

    # ownership: deal multi-occ then single-occ round-robin; sizes <= 128
    own = [[] for _ in range(NCORES)]
    for i, c in enumerate(multi):
        own[i % NCORES].append(c)
    for i, c in enumerate(single):
        own[(i + len(multi)) % NCORES].append(c)
    assert max(len(o) for o in own) <= P

    fill_iter = iter(fill)
    own_full = []
    for r0 in range(NCORES):
        o = list(own[r0])
        while len(o) < P:
            o.append(next(fill_iter))
        own_full.append(o)

    # per-core lane order: tile 0 = own classes; tiles 1.. = other cores' own
    # classes with ALL non-own multi-occ classes first (so rounds >=1 only
    # touch a fixed tile prefix)
    lanes = []
    maxnm = 0
    for r0 in range(NCORES):
        others = []
        for q in range(NCORES):
            if q != r0:
                others.extend(own_full[q])
        om = [c for c in others if len(occ.get(c, ())) > 1]
        os_ = [c for c in others if len(occ.get(c, ())) <= 1]
        maxnm = max(maxnm, len(om))
        lanes.append(own_full[r0] + om + os_)
    RT = 1 + (maxnm + P - 1) // P     # tiles touched by rounds >= 1
    fo_list = [0]
    off = NT
    for r in range(1, R):
        fo_list.append(off)
        off += RT
    NFT = off
    return lanes, occ, kcls, R, RT, NFT, fo_list


def prepare(features, prototypes, labels):
    """Host-side specialization: build the SPMD program and per-core inputs."""
    features = np.asarray(features, dtype=np.float32)
    prototypes = np.asarray(prototypes, dtype=np.float32)
    labels_np = np.asarray(labels).astype(np.int64)

    lanes, occ, kcls, R, RT, NFT, fo_list = _plan(labels_np)
    nc = build_program(NFT, R, RT, fo_list)

    kcls_arr = np.asarray(kcls, dtype=np.int64)
    proto16 = prototypes.astype(np.float16)

    in_maps = []
    for r0 in range(NCORES):
        lane = np.asarray(lanes[r0], dtype=np.int64)      # [CPC]
        # chain inputs: proto rows tile-major (tile t lane p = lane[t*128+p])
        rows = np.zeros((NT + NFT, P, D), dtype=np.float16)
        rows[0:NT] = proto16[lane].reshape(NT, P, D)
        f16 = features.astype(np.float16)
        f0 = np.zeros((CPC, D), dtype=np.float16)
        for L, c in enumerate(lane):
            ts = occ.get(int(c))
            if ts:
                f0[L] = f16[ts[0]]
        rows[NT : 2 * NT] = f0.reshape(NT, P, D)
        for r in range(1, R):
            fr = np.zeros((RT * P, D), dtype=np.float16)
            for L in range(RT * P):
                ts = occ.get(int(lane[L]))
                if ts and len(ts) > r:
                    fr[L] = f16[ts[r]]
            rows[NT + fo_list[r] : NT + fo_list[r] + RT] = fr.reshape(RT, P, D)
        ufg_host = np.ascontiguousarray(rows.transpose(1, 0, 2))  # [P, NT+NFT, D]

        # per-core-rotated K columns (own 896 first), transposed
        krot = np.roll(kcls_arr, -r0 * KC)
        ptTK = np.ascontiguousarray(proto16[krot].T.reshape(2, P, KH))

        # exact fp16 squared norms of own rows; block 7 (chain rows) -> 1.0
        n16 = np.ones((P, NB), dtype=np.float32)
        ownk16 = proto16[krot[: KC]].astype(np.float32)
        n16[:, : NB - 1] = (
            (ownk16 * ownk16).sum(axis=1).reshape(NB - 1, P).T
        )

        in_maps.append({"ufg": ufg_host, "ptTK": ptTK, "n16": n16})

    return nc, in_maps


def kernel(features, prototypes, labels):
    nc, in_maps = prepare(features, prototypes, labels)
    res = run_bass_kernel_spmd(nc, in_maps, list(range(NCORES)))
    total = sum(
        float(np.asarray(res.results[i]["partial"], dtype=np.float64).sum())
        for i in range(NCORES)
    )
    loss = (TEMP / BASE_TEMP) * (total / C)
    return np.asarray(loss, dtype=np.float32)
